# revision 1
# baseline (speedup 1.0000x reference)
"""Sliding-window attention (B=2,S=2048,H=8,D=64,W=128) on 8 trn2 cores, v2.

Sharding: 16 (b,h) pairs -> 8 cores x 2 heads (same b). Host pre-packs per-core
inputs so the device does no transposes:
  qT/kT [128, 2048] fp16  rows bh*64+d, cols seq (host transpose + fp16 cast)
  vp    [128, 2, 16, 65] fp16  partition=k%128, (bh, kb, d) + ones col 64
  mtri  [128, 3, 256] fp16  [:, :, 0:128]=lower-keep tri, [:, :, 128:256]=upper

Device (fp16 on-chip, fp32 accum). Tile tracks PSUM deps per-tile, so scores
and pv live in separate pools:
  scores: per (kb-triple, bh): <=3 matmuls into one of two 3-bank PSUM tiles
          ([128, 3, 512]; strip kb at [:, j, 0:384], section t=kb-1+s at
          cols 128*s); ONE strided-AP exp per group -> st SBUF fp16.
  masks:  two fused DVE multiplies per group over [128, G, 128] sections.
  pv:     per (t, bh): <=3 accumulating matmuls into [128, 65] slots of
          [128, 4, 65] PSUM tiles (pool bufs=2); col 64 = softmax denom.
  norm:   per 4-unit batch: reciprocal [128,4,1] + one broadcast
          tensor_tensor multiply -> osb.
  out:    chunked DMA of osb [128, 16, 128] fp32 -> DRAM [2048, 128].
"""

import numpy as np

B, S, H, D = 2, 2048, 8, 64
W = 128
NB = S // 128  # 16 seq blocks

NWARM = 16  # PE p-state warmup matmuls
GROUPS = [list(range(3 * i, min(3 * i + 3, NB - 1))) for i in range(5)]  # kb 0..14
# kb 15 is handled by a final combined group holding both heads in one tile

_cached = {}


def _install_drain_split():
    """Walrus in this container encodes ~1 sync-wait per CTRL instruction; the
    Tile end-of-kernel drain aggregates one wait per live semaphore and fails
    codegen. Split the waits across single-wait NoOps on the sync engine."""
    import concourse.tile as tile
    from bass_rust import VectorClock, ScopedClock

    def _split_drain_and_barrier(self, tick_clock, wait_clock):
        gc = tick_clock.global_clock
        vals = [gc.peek_next(i) - 1 for i in range(27)]
        for i, v in [(i, v) for i, v in enumerate(vals) if v > 0]:
            sub = VectorClock()
            sub.require_at_least(i, v)
            nop_inst = self.nc.sync.nop(nofuse=True)
            wait_clock.add_sem_waits(nop_inst.ins, ScopedClock({None: sub}))
        self.nc.sync.drain()
        self.nc.all_engine_barrier()
        assert self.sems is not None
        popped = self.nc._tile_sem_poison_stack.pop()
        assert popped is self._sem_poison
        self.nc.clear_and_free_semaphores(list(self.sems.allocated().values()))
        # (no trailing all_engine_barrier: the NEFF completes when every
        # engine's queue drains, so the final cross-engine sync only adds
        # ~200ns to the measured span)

    tile.TileContext._drain_and_barrier = _split_drain_and_barrier


def _build():
    import concourse.bass as bass
    import concourse.mybir as mybir
    import concourse.tile as tile
    from concourse.alu_op_type import AluOpType

    _install_drain_split()

    fp16 = mybir.dt.float16
    fp32 = mybir.dt.float32
    Exp = mybir.ActivationFunctionType.Exp

    # Skip the ~0.7us init all-engine barrier: our first kernel instructions
    # (input DMAs) touch neither the const-AP tensors nor any semaphore the
    # startup gpsimd clears could race with (first sem update lands >2us
    # after the clears; per-engine in-order keeps preamble before body).
    _orig_barrier = bass.Bass.all_engine_barrier
    bass.Bass.all_engine_barrier = lambda self, *a, **k: None
    try:
        nc = bass.Bass()
    finally:
        bass.Bass.all_engine_barrier = _orig_barrier
    q_in = nc.dram_tensor("q", [128, 2048], fp16, kind="ExternalInput")
    k_in = nc.dram_tensor("k", [128, 2048], fp16, kind="ExternalInput")
    hd_in = nc.dram_tensor("hd", [128, 2, 512], fp16, kind="ExternalInput")
    v_in = nc.dram_tensor("v", [128, 2, 16, 65], fp16, kind="ExternalInput")
    m_in = nc.dram_tensor("mtri", [128, 3, 256], fp16, kind="ExternalInput")
    out = nc.dram_tensor("out", [S, 128], fp32, kind="ExternalOutput")

    from contextlib import ExitStack

    with tile.TileContext(nc) as tc, ExitStack() as ctx:
        consts = ctx.enter_context(tc.tile_pool(name="consts", bufs=1))
        ps = ctx.enter_context(tc.tile_pool(name="ps", bufs=1, space="PSUM"))
        pvp = ctx.enter_context(tc.tile_pool(name="pvp", bufs=2, space="PSUM"))
        small = ctx.enter_context(tc.tile_pool(name="small", bufs=8))

        qk = consts.tile([128, 2, 2048], fp16, tag="qk")  # [0]=kT, [1]=qT
        kt = qk[:, 0, :]
        qt = qk[:, 1, :]
        vt = consts.tile([128, 2, 16, 65], fp16, tag="vt")
        mt = consts.tile([128, 3, 256], fp16, tag="mt")
        wk = consts.tile([128, 128], fp16, tag="wk")
        osb = consts.tile([128, 16, 128], fp32, tag="osb")
        stt = [
            consts.tile([128, 3, 384], fp16, tag=f"st{i}", name=f"st{i}")
            for i in range(11)
        ]
        pa = ps.tile([128, 3, 512], fp32, tag="pa")
        pb = ps.tile([128, 3, 512], fp32, tag="pb")
        AB = [pa, pb]

        # input DMAs in consumption order (SP queue / HWDGE); one packed head
        # DMA delivers the first 512 cols of both kT and qT
        nc.sync.dma_start(out=qk[:, :, 0:512], in_=hd_in[:])
        nc.sync.dma_start(out=kt[:, 512:1280], in_=k_in[:, 512:1280])
        nc.sync.dma_start(out=mt, in_=m_in[:])
        nc.sync.dma_start(out=qt[:, 512:1280], in_=q_in[:, 512:1280])
        nc.sync.dma_start(out=vt[:, :, 0:8, :], in_=v_in[:, :, 0:8, :])
        nc.sync.dma_start(out=vt[:, :, 8:16, :], in_=v_in[:, :, 8:16, :])
        nc.sync.dma_start(out=kt[:, 1280:2048], in_=k_in[:, 1280:2048])
        nc.sync.dma_start(out=qt[:, 1280:2048], in_=q_in[:, 1280:2048])

        # kb=0 strips only cover cols 128:384; guard the junk region so the
        # fused exp never reads uninitialized PSUM
        nc.vector.memset(pa[:, 0, 0:128], -1e5)
        nc.vector.memset(pb[:, 0, 0:128], -1e5)

        # PE p-state warmup: anchored on a Pool-engine memset (Pool is idle and
        # past the framework preamble early) so the ramp starts ASAP; outputs
        # go to pb's spare cols which nothing reads
        nc.gpsimd.memset(wk, 0.0)
        for _ in range(NWARM):
            nc.tensor.matmul(
                pb[:, 2, 384:512], wk[:, 0:128], wk[:, 0:128], start=True, stop=True
            )

        out_view = out.rearrange("(t p) c -> p t c", p=128)
        pvts = {}

        def emit_mm(T, rows, j, kb):
            t0, t1 = max(0, kb - 1), min(NB - 1, kb + 1)
            c0 = (t0 - kb + 1) * 128
            w = (t1 - t0 + 1) * 128
            nc.tensor.matmul(
                T[:, j, c0 : c0 + w],
                kt[rows, kb * 128 : (kb + 1) * 128],
                qt[rows, t0 * 128 : t0 * 128 + w],
                start=True,
                stop=True,
            )

        def emit_scores(idx, gq, bh):
            T = AB[idx % 2]
            rows = slice(bh * 64, bh * 64 + 64)
            st = stt[idx]
            grp = GROUPS[gq]
            G = len(grp)
            for j, kb in enumerate(grp):
                emit_mm(T, rows, j, kb)
            nc.scalar.activation(
                out=st[:, 0:G, :], in_=T[:, 0:G, 0:384], func=Exp, scale=0.125
            )
            nc.vector.tensor_tensor(
                out=st[:, 0:G, 0:128], in0=st[:, 0:G, 0:128], in1=mt[:, 0:G, 0:128],
                op=AluOpType.mult,
            )
            nc.vector.tensor_tensor(
                out=st[:, 0:G, 256:384], in0=st[:, 0:G, 256:384],
                in1=mt[:, 0:G, 128:256], op=AluOpType.mult,
            )

        def emit_scores_final(idx):
            # kb=15 for both heads shares one tile (j = head index)
            T = AB[idx % 2]
            st = stt[10]
            for bb in (0, 1):
                rows = slice(bb * 64, bb * 64 + 64)
                nc.tensor.matmul(
                    T[:, bb, 0:256],
                    kt[rows, 15 * 128 : 2048],
                    qt[rows, 14 * 128 : 2048],
                    start=True,
                    stop=True,
                )
            nc.scalar.activation(
                out=st[:, 0:2, 0:256], in_=T[:, 0:2, 0:256], func=Exp, scale=0.125
            )
            # Pool is idle at the end of the exp stream while DVE still has
            # norm batches queued - run the last mask there
            nc.gpsimd.tensor_tensor(
                out=st[:, 0:2, 0:128], in0=st[:, 0:2, 0:128], in1=mt[:, 0:2, 0:128],
                op=AluOpType.mult,
            )

        def emit_pv(t, bb):
            u = 2 * t + bb
            m = u // 4
            if m not in pvts:
                pvts[m] = pvp.tile([128, 4, 65], fp32, tag="pv", name=f"pv{m}")
            slot = pvts[m][:, u % 4, :]
            kbs = [kb for kb in (t - 1, t, t + 1) if 0 <= kb < NB]
            for i2, kb in enumerate(kbs):
                if kb == NB - 1:
                    sti, jj = stt[10], bb
                else:
                    sti, jj = stt[2 * (kb // 3) + bb], kb % 3
                c = (t - kb + 1) * 128
                nc.tensor.matmul(
                    slot,
                    sti[:, jj, c : c + 128],
                    vt[:, bb, kb, :],
                    start=(i2 == 0),
                    stop=(i2 == len(kbs) - 1),
                )

        def emit_norm(m):
            T = pvts[m]
            rt = small.tile([128, 4, 1], fp32, tag="rt", name=f"rt{m}")
            nc.vector.reciprocal(out=rt, in_=T[:, :, 64:65])
            # osb[:, 2m:2m+2, :] viewed as [128, 4, 64] matches T's 4 units
            ov = osb[:, 2 * m : 2 * m + 2, :].rearrange(
                "p a (b c) -> p (a b) c", b=2, c=64
            )
            nc.vector.tensor_tensor(
                out=ov, in0=T[:, :, 0:64], in1=rt.broadcast_to([128, 4, 64]),
                op=AluOpType.mult,
            )

        # pipeline: per kb-triple, both bh score groups, then newly-enabled pv
        # units (t <= 3*gq+1), completed norm batches, ready output chunks
        # gq0's pv units are lagged to after gq1's scores so their mask-waits
        # don't head-of-line block the idx2/idx3 matmuls on PE
        pv_ranges = {0: range(0, 0), 1: range(0, 5), 2: range(5, 8),
                     3: range(8, 11), 4: range(11, 14), 5: range(14, 16)}
        norm_batches = {0: [], 1: [0, 1], 2: [2, 3], 3: [4], 4: [5, 6], 5: [7]}
        out_chunks = {1: [(0, 2)], 2: [(2, 6)], 3: [(6, 10)],
                      4: [(10, 12), (12, 14)], 5: [(14, 16)]}
        for gq in range(6):
            if gq < 5:
                emit_scores(2 * gq + 0, gq, 0)
                emit_scores(2 * gq + 1, gq, 1)
                if gq == 4:
                    # emit the final group's DVE mask ahead of the m5/m6 norm
                    # chain so pv(t14/t15) is not head-of-line blocked
                    emit_scores_final(10)
            for t in pv_ranges[gq]:
                # t15 first in the last block: it does not read the kb15
                # lo-masked region, so it can run during the final Pool mask
                emit_pv(t, 0)
                emit_pv(t, 1)
            for m in norm_batches[gq]:
                emit_norm(m)
            for lo, hi in out_chunks.get(gq, []):
                nc.sync.dma_start(out=out_view[:, lo:hi, :], in_=osb[:, lo:hi, :])

    _spill_excess_waits(nc, mybir, cap=1)
    return nc


def _spill_excess_waits(nc, mybir, cap=1):
    """This walrus build encodes only a couple of sync waits per instruction.
    Move excess waits onto single-wait NoOps inserted just before the victim
    on the same engine queue (thresholds are monotone, so waiting for them
    one-by-one in order is equivalent)."""
    nid = [0]
    for bb in nc.main_func.blocks:
        il = bb.instructions
        new_list = []
        for ins in il:
            si = ins.sync_info
            if si is not None and len(si.on_wait) > cap:
                waits = list(si.on_wait)
                for w in waits[:-cap]:
                    nop = mybir.InstNoOp(name=f"I-spw-{nid[0]}", ins=[], outs=[])
                    nid[0] += 1
                    nop.engine = ins.engine
                    nop.sync_info = mybir.SyncInfo(on_wait=[w], on_update=[])
                    new_list.append(nop)
                ins.sync_info = mybir.SyncInfo(
                    on_wait=waits[-cap:], on_update=list(si.on_update)
                )
            new_list.append(ins)
        il[:] = new_list


def kernel(query, key, value, window_size):
    assert int(window_size) == W
    from concourse.bass_utils import run_bass_kernel_spmd

    if "nc" not in _cached:
        _cached["nc"] = _build()
    nc = _cached["nc"]

    kk, qq = np.arange(128)[:, None], np.arange(128)[None, :]
    mtri = np.zeros((128, 3, 256), np.float16)
    mtri[:, :, 0:128] = (kk <= qq)[:, None, :]
    mtri[:, :, 128:256] = (kk >= qq)[:, None, :]

    q = np.asarray(query, np.float32)
    k = np.asarray(key, np.float32)
    v = np.asarray(value, np.float32)
    in_maps = []
    for c in range(8):
        b, h0 = c // 4, 2 * (c % 4)
        qc = q[b, :, h0 : h0 + 2, :]  # [S, 2, 64]
        kc = k[b, :, h0 : h0 + 2, :]
        vc = v[b, :, h0 : h0 + 2, :]
        qT = np.ascontiguousarray(qc.transpose(1, 2, 0).reshape(128, S)).astype(
            np.float16
        )
        kT = np.ascontiguousarray(kc.transpose(1, 2, 0).reshape(128, S)).astype(
            np.float16
        )
        vp = np.ones((128, 2, 16, 65), np.float16)
        vp[:, :, :, 0:64] = vc.reshape(16, 128, 2, 64).transpose(1, 2, 0, 3)
        hd = np.stack([kT[:, 0:512], qT[:, 0:512]], axis=1)  # [128, 2, 512]
        in_maps.append({"q": qT, "k": kT, "hd": hd, "v": vp, "mtri": mtri})

    res = run_bass_kernel_spmd(nc, in_maps, list(range(8)))
    full = np.empty((B, S, H, D), np.float32)
    for c in range(8):
        b, h0 = c // 4, 2 * (c % 4)
        o = res.results[c]["out"]  # [S, 128]
        full[b, :, h0 : h0 + 2, :] = o.reshape(S, 2, D)
    return full

